# revision 41
# baseline (speedup 1.0000x reference)
"""Trainium2 Bass kernel for CausalSelfAttention (GQA + RoPE).

Sharding: tensor-parallel over heads across 8 cores (2 q heads + 1 kv
head per core); host sums the 8 partial output projections.

fp8 DoubleRow (vs the bf16 kernel, 259.3us -> target ~205us):
  - MM1 and MM4 run as compensated fp8e4 DoubleRow matmuls: each
    operand is split hi+lo (same power-of-2 scale; lo = fp8(a - hi),
    so hi+lo carries ~0.2% error, bf16-grade). Per ko-PAIR of 128,
    three DoubleRow matmuls (hi@hi, lo@hi, hi@lo at 0.5 cyc/row each)
    replace two bf16 matmuls (1.0 cyc/row): 25% fewer PE cycles at
    measured max-rel 3.6e-3 (sim) vs the 2e-2 gate.
  - scales: x*8, w_attn*128, w_proj*128, all powers of 2 folded out
    exactly: qkv planes carry 1024x, exp scale /2^20, ones_col=128
    makes the normalized y carry 8x (prime fp8 range, residual clears
    subnormals), MM4 psum carries 1024x, host divides once.
  - normalize emits ytc hi (Pool copy) + lo (DVE sub) fp8 pair.
  - attention (scores/PV/denominator) stays bf16: full-fp8 there
    measured 1.7-4.5e-2 -- over or too near the gate.

Design (vs the fp32r baseline, 344.6us -> 259.3us):
  - bf16 end-to-end in SBUF/DRAM (fp32 only in PSUM accumulation):
    halves DMA bytes, removes the fp32r small-moving-dim 4x penalty.
    Validated numerics: rel err 4.3e-3 vs the 2e-2 gate.
  - chunky DMAs (~70 vs 405): the per-DMA ~650ns HWDGE/SP issue cost
    made the baseline sequencer-bound on DMA issue.
  - RoPE rotate-half without PE: the host permutes q/k head dims to
    even-first and folds the rotation sign into sin, so rotate-half is
    a partition half-swap done by two SBUF->SBUF DMAs issued right
    after each qkv eviction; all rope muls are cheap all-SBUF bf16 DVE
    ops (no J-matmul, no PSUM reads).
  - PSUM: 4-buffer "flex" pool (MM1 acc / transposes / scores /
    MM4 acc) + two 2-buffer banks that alternate between PV-accum and
    denominator roles each attention chunk, so a new chunk's first
    matmul lands on the earliest-released bank.
  - denominator: full tk-tiles pre-summed on DVE in groups of 8 (bf16
    2x mode), one accumulated ones-matmul per group; diagonal tiles
    accumulate directly on PE but are deferred one ti-step so the
    exp -> affine_select mask latency never stalls the PE.
  - attention cq0 runs interleaved inside phase A (MM1 fills its exp
    bubbles); each chunk's RoPE/v-transpose stage is pipelined into
    the next chunk's MM1; MM4 work flows through a global fill queue
    at ~1 item per ti-step, also absorbed by batch 1's phase A.
  - scores are software-pipelined one ti-step ahead; MM4 evictions
    split ~3/4 DVE / 1/4 ACT (ACT is the exp pacer).
  - PE p-state warmup during the initial DMA wait; ko-group-major
    first chunk starts compute on 1/8 of the first DMA.
"""

import math

import numpy as np

B, T, C = 2, 2048, 2048
NH = 16
NKV = 8
HD = 128
NCORES = 8
NQ = NH // NCORES   # 2 query heads per core
P = 128
TCH = 512           # phase-A token chunk
QCH = 512           # attention tq chunk (1 PSUM bank)
MMF = 512           # matmul moving free dim

_CACHE = {}


def _build_nc():
    import concourse.bacc as bacc
    import concourse.mybir as mybir
    from concourse.tile import TileContext
    from concourse.masks import make_identity

    F32 = mybir.dt.float32
    BF16 = mybir.dt.bfloat16
    F8 = mybir.dt.float8e4
    DR = mybir.MatmulPerfMode.DoubleRow
    AF = mybir.ActivationFunctionType
    ALU = mybir.AluOpType

    KO = C // P            # 16 contraction tiles for MM1
    KP = KO // 2           # 8 DoubleRow ko-pairs
    NT = NQ + 2            # 4 n-tiles (q0,q1,k,v)
    NL = NT * HD           # 512 local qkv cols
    TT = T // P            # 16 tk tiles per batch
    NCH = T // TCH         # 4 phase-A chunks per batch
    NQC = T // QCH         # 4 attention chunks per batch
    KTQ = QCH // P         # 4 tk-tiles per attention chunk step
    CCH = C // MMF         # 4 output column chunks
    GCN = B * NCH          # 8 global chunks
    scale = 1.0 / math.sqrt(HD)
    # x carries 8x, w_attn 128x -> raw qkv planes carry 1024x; cos/sin
    # carry 2^-10 so the fp8 q/k planes are true-scale; v keeps 1024x,
    # ones_col=128 makes the normalized y carry 8x; w_proj 128x -> out
    # carries 1024x (host divides once).
    EXPS = scale

    nc = bacc.Bacc("TRN2", target_bir_lowering=False)
    # {hi, lo} fp8 split on a middle dim (same scale per tensor); xt is
    # pre-chunked on the host so each chunk's (hi/lo, t) is contiguous
    # and the DMA balancer can merge the inner dims.
    xt = nc.dram_tensor("xt", [B * T // TCH, C, 2, TCH], F8,
                        kind="ExternalInput")
    wa = nc.dram_tensor("wa", [C, 2, NL], F8, kind="ExternalInput")
    wp = nc.dram_tensor("wp", [NQ * HD, 2, C], F8, kind="ExternalInput")
    cs = nc.dram_tensor("cs", [P, T], BF16, kind="ExternalInput")
    sn = nc.dram_tensor("sn", [P, T], BF16, kind="ExternalInput")
    out = nc.dram_tensor("out", [B * T, C], BF16, kind="ExternalOutput")

    with TileContext(nc) as tc:
        with tc.tile_pool(name="const", bufs=1) as constp, \
             tc.tile_pool(name="xtp", bufs=3) as xtp, \
             tc.tile_pool(name="plane", bufs=1) as planep, \
             tc.tile_pool(name="work", bufs=3) as workp, \
             tc.tile_pool(name="grpp", bufs=8) as grpp, \
             tc.tile_pool(name="small", bufs=2) as smallp, \
             tc.tile_pool(name="expp", bufs=6) as expp, \
             tc.tile_pool(name="ytcp", bufs=3) as ytcp, \
             tc.tile_pool(name="outp", bufs=4) as outp, \
             tc.tile_pool(name="ps", bufs=1, space="PSUM") as ps:

            # PSUM budget (8 banks): flex(4) + accA(2) + accB(2).
            # flex serves MM1 accumulators, rope jp, v transposes,
            # scores (sps) and MM4 accumulators; accA/accB alternate
            # between yacc and dacc roles per attention chunk so a new
            # chunk's yacc lands on the earliest-released banks.
            def mm_tile():
                return ps.tile([P, MMF], F32, tag="flex", name="flex",
                               bufs=4)

            sps_tile = mm_tile

            def acc_tile(par, nm):
                return ps.tile([P, QCH], F32, tag=f"acc{par}", name=nm,
                               bufs=2)

            cq_counter = [0]

            # PE warmup: dummy matmuls during the initial DMA wait keep
            # the tensor engine "continuously busy" so the p-state model
            # has it at full clock when real work arrives (~3.6us in).
            dummy = constp.tile([P, P], BF16, tag="dummy", name="dummy")
            nc.vector.memset(dummy, 0.0)
            for _ in range(36):
                nc.tensor.matmul(mm_tile()[:, :P], dummy, dummy,
                                 start=True, stop=True)

            # ---------------- constants + first chunks' DMA ------------
            wa_sb = constp.tile([P, KO, 2, NL], F8, tag="wa", name="wa")
            wa_r = wa.rearrange("(ko p) two n -> p ko two n", p=P)
            xt_r = xt.rearrange("g (ko p) two t -> g p ko two t", p=P)
            xt_tiles = {}
            xt0 = xtp.tile([P, KO, 2, TCH], F8, tag="xt", name="xt0")
            C0G = [(0, 2), (2, 4), (4, 6), (6, 8), (8, 10), (10, 12), (12, 16)]
            for (g0, g1) in C0G:
                nc.sync.dma_start(wa_sb[:, g0:g1], wa_r[:, g0:g1])
                nc.sync.dma_start(xt0[:, g0:g1], xt_r[0, :, g0:g1])
            xt_tiles[0] = xt0

            def emit_xt_dma(ci):
                tile = xtp.tile([P, KO, 2, TCH], F8, tag="xt",
                                name=f"xt{ci}")
                nc.sync.dma_start(tile[:, :KO // 2],
                                  xt_r[ci, :, :KO // 2])
                nc.sync.dma_start(tile[:, KO // 2:],
                                  xt_r[ci, :, KO // 2:])
                xt_tiles[ci] = tile

            def mm1_mms(acc, kp, n, xt_sb, start, stop, skip=False):
                """3 compensated DoubleRow matmuls for one ko-pair."""
                wh = wa_sb[:, 2 * kp:2 * kp + 2, 0, n * P:(n + 1) * P]
                wl = wa_sb[:, 2 * kp:2 * kp + 2, 1, n * P:(n + 1) * P]
                xh = xt_sb[:, 2 * kp:2 * kp + 2, 0]
                xl = xt_sb[:, 2 * kp:2 * kp + 2, 1]
                nc.tensor.matmul(acc, wh, xh, start=start, stop=False,
                                 perf_mode=DR, skip_group_check=skip)
                nc.tensor.matmul(acc, wl, xh, start=False, stop=False,
                                 perf_mode=DR, skip_group_check=skip)
                nc.tensor.matmul(acc, wh, xl, start=False, stop=stop,
                                 perf_mode=DR, skip_group_check=skip)

            cs_sb = constp.tile([P, T], BF16, tag="cs", name="cs")
            sn_sb = constp.tile([P, T], BF16, tag="sn", name="sn")
            nc.sync.dma_start(cs_sb[:, 0:TCH], cs[:, 0:TCH])
            nc.sync.dma_start(sn_sb[:, 0:TCH], sn[:, 0:TCH])
            emit_xt_dma(1)

            wp_sb = constp.tile([P, NQ, 2, C], F8, tag="wp", name="wp")
            wp_r = wp.rearrange("(ko p) two c -> p ko two c", p=P)

            ident_f = constp.tile([P, P], F32, tag="ident_f",
                                  name="ident_f")
            make_identity(nc, ident_f)
            ident = constp.tile([P, P], BF16, tag="ident", name="ident")
            nc.vector.tensor_copy(ident, ident_f)
            ones_f = constp.tile([P, 1], F32, tag="ones_f", name="ones_f")
            nc.vector.memset(ones_f, 128.0)  # folds v's 1024x into y=8x
            ones_col = constp.tile([P, 1], BF16, tag="ones_col",
                                   name="ones_col")
            nc.vector.tensor_copy(ones_col, ones_f)

            # ---------------- per-batch plane state ----------------
            q_planes = {}
            k_plane = {}
            v_nat = {}

            def alloc_planes(b):
                # q/k planes are fp8 in hd-split layout [64, 2, TCH]:
                # [:, 0] = rope dims 0..63, [:, 1] = dims 64..127, so
                # scores run as one DoubleRow matmul (2x64 contraction).
                # One tile PER CHUNK: the planes are DMA-written, and
                # per-chunk tiles keep the read deps exact.
                q_planes[b] = [[planep.tile([64, 2, TCH], F8,
                                            tag=f"qk{h}c{c}",
                                            name=f"q{h}b{b}c{c}", bufs=2)
                                for c in range(NCH)]
                               for h in range(NQ)]
                k_plane[b] = [planep.tile([64, 2, TCH], F8,
                                          tag=f"kplc{c}",
                                          name=f"kb{b}c{c}", bufs=2)
                              for c in range(NCH)]
                v_nat[b] = planep.tile([P, TT, HD], BF16, tag="vnat",
                                       name=f"vb{b}", bufs=2)

            # ---------------- MM4 (output projection) ----------------
            osb_hold = {}
            osb_mode = {}

            def emit_mm4(ytc_prev, row0, tl, cc, par,
                         half_dma=False):
                if cc == 0:
                    osb_hold[tl] = outp.tile([P, C], BF16, tag="o",
                                             name="o")
                    osb_mode[tl] = half_dma
                half_dma = osb_mode[tl]
                osb = osb_hold[tl]
                oacc = mm_tile()
                ytch, ytcl = ytc_prev
                ccs = slice(cc * MMF, (cc + 1) * MMF)
                yh = ytch[:, :, tl:tl + P]
                yl = ytcl[:, :, tl:tl + P]
                # lo-term last: gives the normalize sub (ytcl) the most
                # slack before the PE needs it
                nc.tensor.matmul(oacc, yh, wp_sb[:, :, 0, ccs],
                                 start=True, stop=False, perf_mode=DR)
                nc.tensor.matmul(oacc, yh, wp_sb[:, :, 1, ccs],
                                 start=False, stop=False, perf_mode=DR)
                nc.tensor.matmul(oacc, yl, wp_sb[:, :, 0, ccs],
                                 start=False, stop=True, perf_mode=DR)
                sl = osb[:, cc * MMF:(cc + 1) * MMF]
                # drain groups rotate evict engines per cc so the
                # tail's evictions pipeline across ACT+DVE+Pool;
                # mid-stream groups keep ACT at ~1/4 share (ACT is the
                # exp pacer)
                if half_dma:
                    eng = (nc.scalar.copy, nc.vector.tensor_copy,
                           nc.gpsimd.tensor_copy)[cc % 3]
                    eng(sl, oacc)
                elif par % 4 == 3:
                    nc.scalar.copy(sl, oacc)
                else:
                    nc.vector.tensor_copy(sl, oacc)
                # final-drain groups use half-tile DMAs (after cc1 and
                # cc3) so the first half transfers while the second
                # half computes; mid-stream groups use one full DMA.
                # The mode is latched per group at cc0 so a group that
                # straddles the drain boundary stays consistent.
                if half_dma == "cc":
                    nc.sync.dma_start(
                        out[row0:row0 + P, cc * MMF:(cc + 1) * MMF], sl)
                elif half_dma and cc in (1, CCH - 1):
                    h0c = 0 if cc == 1 else C // 2
                    nc.sync.dma_start(
                        out[row0:row0 + P, h0c:h0c + C // 2],
                        osb[:, h0c:h0c + C // 2])
                elif not half_dma and cc == CCH - 1:
                    nc.sync.dma_start(out[row0:row0 + P, :], osb)

            # global MM4 fill queue: attention pulls ~1-2 items per
            # ti-step; items flow across chunk boundaries
            fill_q = []

            def fill_push(ytc_prev, base):
                par = len(fill_q)
                for tl in range(0, QCH, P):
                    for cc in range(CCH):
                        fill_q.append((ytc_prev, base + tl, tl, cc,
                                       par))
                        par += 1

            def fill_pull(nmax):
                n = 0
                while fill_q and n < nmax:
                    emit_mm4(*fill_q.pop(0))
                    n += 1

            # ---------------- attention emitters ----------------
            class AttState:
                pass

            def att_begin(b, cq):
                st = AttState()
                st.b, st.cq = b, cq
                st.tq0 = cq * QCH
                st.ntk = KTQ * (cq + 1)
                par = cq_counter[0] % 2
                cq_counter[0] += 1
                st.yaccs = [acc_tile(par, f"yac{h}") for h in range(NQ)]
                st.daccs = [acc_tile(1 - par, f"dac{h}")[:1, :]
                            for h in range(NQ)]
                st.grps = [[None] * ((cq * KTQ + 7) // 8)
                           for _ in range(NQ)]
                st.exs = {}
                st.deferred = []
                st.ytch = ytcp.tile([P, NQ, QCH], F8, tag="ytch",
                                    name="ytch")
                st.ytcl = ytcp.tile([P, NQ, QCH], F8, tag="ytcl",
                                    name="ytcl")
                return st

            def att_drain_deferred(st):
                """Masked-diagonal matmuls, deferred one ti-step so the
                exp->affine_select latency never stalls the PE. Only
                start=False accumulations may be deferred: a start=True
                matmul resets its PSUM bank, so it must stay the first
                write (handled inline in att_fin)."""
                for (ti, h, ex, o, last, do_y) in st.deferred:
                    if do_y:
                        nc.tensor.matmul(
                            st.yaccs[h][:, o:o + P], v_nat[st.b][:, ti],
                            ex[:, o:o + P], start=False, stop=last,
                            skip_group_check=True)
                    nc.tensor.matmul(
                        st.daccs[h][:, o:], ones_col, ex[:, o:],
                        start=(ti == KTQ * st.cq),
                        stop=(st.cq == 0 and last),
                        skip_group_check=True)
                st.deferred = []

            def att_sps(st, ti, h):
                o = max(0, (ti - KTQ * st.cq) * P)
                sps = sps_tile()
                kc = (ti % KTQ) * P
                nc.tensor.matmul(
                    sps[:, o:],
                    k_plane[st.b][ti // KTQ][:, :, kc:kc + P],
                    q_planes[st.b][h][st.cq][:, :, o:QCH],
                    start=True, stop=True, perf_mode=DR)
                ex = expp.tile([P, QCH], BF16, tag="exp", name="exp")
                nc.scalar.activation(ex[:, o:], sps[:, o:], AF.Exp,
                                     scale=EXPS)
                if ti >= KTQ * st.cq:  # diagonal tile: mask tq < tk
                    nc.gpsimd.affine_select(
                        ex[:, o:o + P], ex[:, o:o + P],
                        pattern=[[1, P]], compare_op=ALU.is_ge,
                        fill=0.0, base=0, channel_multiplier=-1)
                st.exs[(ti, h)] = ex

            def att_fin(st, ti, h):
                o = max(0, (ti - KTQ * st.cq) * P)
                first, last = (ti == 0), (ti == st.ntk - 1)
                ex = st.exs.pop((ti, h))
                if ti < KTQ * st.cq:
                    nc.tensor.matmul(
                        st.yaccs[h][:, o:], v_nat[st.b][:, ti],
                        ex[:, o:], start=first, stop=last,
                        skip_group_check=True)
                    # full tile: accumulate into its group-of-8 sum
                    eng = nc.vector
                    gi = ti // 8
                    if ti % 8 == 0:
                        g = grpp.tile([P, QCH], BF16, tag="grp",
                                      name="grp")
                        eng.tensor_copy(g, ex)
                        st.grps[h][gi] = g
                    else:
                        g = st.grps[h][gi]
                        eng.tensor_add(g, g, ex)
                elif first:
                    # cq0's first tile: single start=True write for the
                    # whole bank (start resets the bank, so it cannot be
                    # split); waits for the affine mask, but the fine
                    # interleave absorbs that
                    nc.tensor.matmul(
                        st.yaccs[h][:, o:], v_nat[st.b][:, ti],
                        ex[:, o:], start=True, stop=False,
                        skip_group_check=True)
                    st.deferred.append((ti, h, ex, o, last, False))
                else:
                    # diagonal tile: the unmasked suffix can run now;
                    # the masked 128-wide block + denominator defer one
                    # ti-step (see att_drain_deferred)
                    if o + P < QCH:
                        nc.tensor.matmul(
                            st.yaccs[h][:, o + P:],
                            v_nat[st.b][:, ti], ex[:, o + P:],
                            start=False, stop=False,
                            skip_group_check=True)
                    st.deferred.append((ti, h, ex, o, last, True))
                if last:
                    att_drain_deferred(st)
                    # close the denominator (group matmuls) and start
                    # this head's normalize chain immediately
                    for gi in range(len(st.grps[h])):
                        nc.tensor.matmul(
                            st.daccs[h], ones_col, st.grps[h][gi],
                            start=False,
                            stop=(gi == len(st.grps[h]) - 1),
                            skip_group_check=True)
                    rec_f = smallp.tile([1, QCH], F32, tag="rec_f",
                                        name="rec_f")
                    nc.vector.reciprocal(rec_f, st.daccs[h])
                    bcs = workp.tile([P, QCH], F32, tag="bcast",
                                     name="bcast")
                    nc.gpsimd.partition_broadcast(bcs, rec_f)
                    # m = 8*y (fp32) -> hi (Pool) + lo (DVE) fp8 pair
                    mn = workp.tile([P, QCH], F32, tag="mnorm",
                                    name="mnorm")
                    nc.vector.tensor_mul(mn, st.yaccs[h], bcs)
                    nc.gpsimd.tensor_copy(st.ytch[:, h], mn)
                    nc.vector.tensor_sub(st.ytcl[:, h], mn,
                                         st.ytch[:, h])

            def attention_steps(b, cq, fills=0, prelude=None,
                                fine=False):
                """Yields after each sub-phase; last yield is the state
                (with .ytc set). fine=True yields after every single
                emission (for interleaving into phase A). With fills,
                pulls `fills` items/ti from the global queue, and scores
                are software-pipelined one ti-step ahead."""
                st = att_begin(b, cq)
                if not fine:
                    att_sps(st, 0, 0)
                    att_sps(st, 0, 1)
                    yield None
                for ti in range(st.ntk):
                    if fine:
                        att_sps(st, ti, 0)
                        yield None
                        att_sps(st, ti, 1)
                        yield None
                    if prelude is not None:
                        next(prelude, None)
                    if fills:
                        # ti=0 pulls would hit items whose normalize
                        # chain just started; defer them to the tail
                        if ti == 0:
                            pass
                        elif ti >= st.ntk - 2:
                            fill_pull(fills + 1)
                        else:
                            fill_pull(fills)
                    if not fine and ti + 1 < st.ntk:
                        att_sps(st, ti + 1, 0)
                        att_sps(st, ti + 1, 1)
                    if ti < st.ntk - 1:
                        att_drain_deferred(st)
                    att_fin(st, ti, 0)
                    if fine:
                        yield None
                    att_fin(st, ti, 1)
                    yield None
                yield st

            # ---------------- phase A ----------------
            def phase_a_prologue(b, c):
                gc = b * NCH + c
                if b == 0 and c + 1 < NCH:
                    t1 = (c + 1) * TCH
                    nc.sync.dma_start(cs_sb[:, t1:t1 + TCH],
                                      cs[:, t1:t1 + TCH])
                    nc.sync.dma_start(sn_sb[:, t1:t1 + TCH],
                                      sn[:, t1:t1 + TCH])
                if b == 0 and c == NCH - 1:
                    for ko in range(NQ):
                        nc.sync.dma_start(wp_sb[:, ko], wp_r[:, ko])

            def mm1_steps(b, c, first_chunk=False):
                """Generator: per n-step, the 16-ko MM1 chain + evict."""
                phase_a_prologue(b, c)
                gc = b * NCH + c
                xt_sb = xt_tiles.pop(gc)
                raw2 = workp.tile([P, NQ, TCH], BF16, tag="raw2",
                                  name="raw2")
                rawk = workp.tile([P, TCH], BF16, tag="rawk", name="rawk")
                vTc = workp.tile([P, TCH], BF16, tag="vTc", name="vTc")
                tmps = {}

                def evict(n, acc):
                    if n < NQ:
                        nc.scalar.copy(raw2[:, n], acc)
                        src_ap = raw2
                        tmps[n] = workp.tile([P, TCH], BF16, tag="tmp",
                                             name="tmp", bufs=6)
                        nc.sync.dma_start(tmps[n][:P // 2],
                                          raw2[P // 2:, n])
                        nc.sync.dma_start(tmps[n][P // 2:],
                                          raw2[:P // 2, n])
                    elif n == NQ:
                        nc.scalar.copy(rawk, acc)
                        tmps[n] = workp.tile([P, TCH], BF16, tag="tmp",
                                             name="tmp", bufs=6)
                        nc.sync.dma_start(tmps[n][:P // 2],
                                          rawk[P // 2:])
                        nc.sync.dma_start(tmps[n][P // 2:],
                                          rawk[:P // 2])
                    else:
                        # v evict on Pool: keeps ACT free for exp and
                        # decouples the v-transpose from the exp stream
                        nc.gpsimd.tensor_copy(vTc, acc)

                if first_chunk:
                    # ko-group-major over the 4 flex banks: PE starts
                    # after the first 2-ko slice of the wa/xt DMA.
                    accs = [mm_tile() for _ in range(NT)]
                    for (g0, g1) in C0G:
                        for n in range(NT):
                            for kp in range(g0 // 2, g1 // 2):
                                mm1_mms(accs[n], kp, n, xt_sb,
                                        start=(kp == 0),
                                        stop=(kp == KP - 1), skip=True)
                        yield None
                    for n in range(NT):
                        evict(n, accs[n])
                else:
                    for n in range(NT):
                        acc = mm_tile()
                        for kp in range(KP):
                            mm1_mms(acc, kp, n, xt_sb,
                                    start=(kp == 0),
                                    stop=(kp == KP - 1))
                        evict(n, acc)
                        yield None
                # xt prefetch issued AFTER this chunk's evict/tmp DMAs:
                # keeps the small latency-critical transfers (tmps, fp8
                # planes) ahead of the next 2MB stream in the queues
                if gc + 2 < GCN:
                    emit_xt_dma(gc + 2)
                yield (raw2, rawk, vTc, tmps)

            def rope_steps(b, c, raw2, rawk, vTc, tmps):
                """Generator: 4 steps: rope(q0), rope(q1), rope(k),
                v transposes. The half-swapped tmps were produced by
                DMA right after each eviction in mm1_steps."""
                t0 = c * TCH

                def rope_one(src, tmp, dst):
                    t2 = workp.tile([P, TCH], BF16, tag="t2", name="t2")
                    nc.vector.tensor_mul(t2, tmp, sn_sb[:, t0:t0 + TCH])
                    t1 = workp.tile([P, TCH], BF16, tag="t1", name="t1")
                    nc.vector.tensor_mul(t1, src, cs_sb[:, t0:t0 + TCH])
                    tb = workp.tile([P, TCH], BF16, tag="tb", name="tb",
                                    bufs=4)
                    nc.vector.tensor_add(tb, t1, t2)
                    # fp8 convert (Pool) + partition half-swap into the
                    # hd-split [64, 2, T] plane (2 SBUF->SBUF DMAs)
                    f8p = workp.tile([P, TCH], F8, tag="f8p", name="f8p",
                                     bufs=4)
                    nc.gpsimd.tensor_copy(f8p, tb)
                    nc.sync.dma_start(dst[:, 0, :], f8p[:P // 2])
                    nc.sync.dma_start(dst[:, 1, :], f8p[P // 2:])

                for h in range(NQ):
                    rope_one(raw2[:, h], tmps[h], q_planes[b][h][c])
                    yield None
                rope_one(rawk, tmps[NQ], k_plane[b][c])
                yield None
                for i in range(KTQ):
                    pt = mm_tile().bitcast(BF16)[:, :P]
                    nc.tensor.transpose(
                        pt, vTc[:, i * P:(i + 1) * P], ident)
                    nc.vector.tensor_copy(v_nat[b][:, t0 // P + i], pt)
                    if i % 2 == 1:
                        yield None

            def phase_a_batch(b, start_slot, pull_aux, fills=False):
                """Run phase A of batch b with chunk-pipelined rope and
                aux pulls (2 per slot from start_slot on). Returns the
                final chunk's rope generator (not drained)."""
                slot = [0]

                def slot_tick():
                    slot[0] += 1
                    if fills:
                        fill_pull(1)
                    if slot[0] >= start_slot:
                        pull_aux()
                        pull_aux()

                prev_rope = None
                for c in range(NCH):
                    mm1 = mm1_steps(b, c, first_chunk=(b == 0 and c == 0))
                    tail = None
                    for v in mm1:
                        if v is not None:
                            tail = v
                            break
                        if prev_rope is not None:
                            next(prev_rope, None)
                        slot_tick()
                    if prev_rope is not None:
                        for _ in prev_rope:
                            slot_tick()
                    prev_rope = rope_steps(b, c, *tail)
                return prev_rope

            # ================= emission schedule =================
            att_cq0 = {}
            att_cq0_done = {}

            def make_cq0_puller(b):
                gen = attention_steps(b, 0, fine=True)
                att_cq0[b] = gen

                def pull():
                    v = next(gen, False)
                    if v is not False and v is not None:
                        att_cq0_done[b] = v
                return pull

            def drain_cq0(b, rope_tail):
                while b not in att_cq0_done:
                    v = next(att_cq0[b], False)
                    if v is False:
                        break
                    if v is not None:
                        att_cq0_done[b] = v
                    if rope_tail is not None:
                        next(rope_tail, None)

            def chain(*its):
                for it in its:
                    yield from it

            def run_attention(b, cq, prelude=None, fills=1):
                st = None
                for v in attention_steps(b, cq, fills=fills,
                                         prelude=prelude):
                    if v is not None:
                        st = v
                fill_push((st.ytch, st.ytcl), b * T + cq * QCH)

            # ---- batch 0 ----
            alloc_planes(0)
            pull0 = make_cq0_puller(0)
            rope_tail = phase_a_batch(0, 14, pull0)
            drain_cq0(0, rope_tail)
            fill_push((att_cq0_done[0].ytch, att_cq0_done[0].ytcl), 0)

            run_attention(0, 1, prelude=rope_tail)
            run_attention(0, 2)
            run_attention(0, 3)

            # ---- batch 1 ----
            alloc_planes(1)
            pull1 = make_cq0_puller(1)
            rope_tail = phase_a_batch(1, 10, pull1, fills=True)
            drain_cq0(1, rope_tail)
            fill_push((att_cq0_done[1].ytch, att_cq0_done[1].ytcl), T)

            run_attention(1, 1, prelude=rope_tail)
            run_attention(1, 2)
            run_attention(1, 3)
            while fill_q:
                # last 2 row-groups: per-cc DMAs so the final transfers
                # start right after each eviction instead of pairing up
                mode = "cc" if len(fill_q) <= 2 * CCH else True
                emit_mm4(*fill_q.pop(0), half_dma=mode)

    nc.finalize()
    return nc


def _host_prep(x, w_attn, w_proj, freqs_cos, freqs_sin):
    """Shard + relayout inputs for the 8 cores (head-parallel).

    x/w_attn/w_proj ship as fp8e4 hi+lo pairs (same pow2 scale: hi =
    fp8(a*s), lo = fp8(a*s - hi)), stacked on dim1: [rows, 2, cols]."""
    import ml_dtypes
    BF = ml_dtypes.bfloat16
    F8 = ml_dtypes.float8_e4m3

    def split8(a, sc):
        a = np.ascontiguousarray(a * sc)
        hi = a.astype(F8)
        lo = (a - hi.astype(np.float32)).astype(F8)
        return np.ascontiguousarray(np.stack([hi, lo], axis=1))

    x = np.asarray(x, dtype=np.float32)
    w_attn = np.asarray(w_attn, dtype=np.float32)
    w_proj = np.asarray(w_proj, dtype=np.float32)
    fc = np.asarray(freqs_cos, dtype=np.float32)
    fs = np.asarray(freqs_sin, dtype=np.float32)

    # [C, 2, B*T] fp8 -> pre-chunked [B*T/TCH, C, 2, TCH]
    xt = split8(x.reshape(B * T, C).T, 8.0)
    xt = np.ascontiguousarray(
        xt.reshape(C, 2, B * T // TCH, TCH).transpose(2, 0, 1, 3))
    # head-dim layout for q/k is permuted to even-dims-first so that
    # rotate-half becomes a partition half-swap on device; the rotation
    # sign is folded into the sin tensor (top half negated)
    perm = np.concatenate([np.arange(0, HD, 2), np.arange(1, HD, 2)])
    cs_i = np.repeat(fc, 2, axis=1).T  # [HD, T] interleaved layout
    sn_i = np.repeat(fs, 2, axis=1).T
    # cos/sin carry 2^-10 to descale the 1024x raw q/k during rope
    cs = np.ascontiguousarray(cs_i[perm] / 1024.0).astype(BF)
    sgn = np.where(np.arange(HD) < HD // 2, -1.0, 1.0)[:, None]
    sn = np.ascontiguousarray(sn_i[perm] * sgn / 1024.0).astype(
        np.float32).astype(BF)

    in_maps = []
    for g in range(NCORES):
        q0 = w_attn[:, 2 * g * HD:(2 * g + 1) * HD][:, perm]
        q1 = w_attn[:, (2 * g + 1) * HD:(2 * g + 2) * HD][:, perm]
        k_cols = w_attn[:, NH * HD + g * HD:
                        NH * HD + (g + 1) * HD][:, perm]
        v_cols = w_attn[:, (NH + NKV) * HD + g * HD:
                        (NH + NKV) * HD + (g + 1) * HD]
        wa_g = split8(
            np.concatenate([q0, q1, k_cols, v_cols], axis=1), 128.0)
        wp_g = split8(w_proj[2 * g * HD:(2 * g + 2) * HD, :], 128.0)
        in_maps.append({"xt": xt, "wa": wa_g, "wp": wp_g,
                        "cs": cs, "sn": sn})
    return in_maps


def kernel(x, w_attn, w_proj, freqs_cos, freqs_sin):
    from concourse.bass_utils import run_bass_kernel_spmd

    if "nc" not in _CACHE:
        _CACHE["nc"] = _build_nc()
    nc = _CACHE["nc"]
    in_maps = _host_prep(x, w_attn, w_proj, freqs_cos, freqs_sin)
    res = run_bass_kernel_spmd(nc, in_maps, core_ids=list(range(NCORES)))
    acc = np.zeros((B * T, C), dtype=np.float64)
    for r in res.results:
        acc += np.asarray(r["out"], dtype=np.float64)
    acc *= 1.0 / 1024.0  # fold out the 8x (y) * 128x (w_proj) scales
    return acc.reshape(B, T, C).astype(np.float32)



# revision 43
# speedup vs baseline: 1.0022x; 1.0022x over previous
"""Trainium2 Bass kernel for CausalSelfAttention (GQA + RoPE).

Sharding: tensor-parallel over heads across 8 cores (2 q heads + 1 kv
head per core); host sums the 8 partial output projections.

fp8 DoubleRow (vs the bf16 kernel, 259.3us -> target ~205us):
  - MM1 and MM4 run as compensated fp8e4 DoubleRow matmuls: each
    operand is split hi+lo (same power-of-2 scale; lo = fp8(a - hi),
    so hi+lo carries ~0.2% error, bf16-grade). Per ko-PAIR of 128,
    three DoubleRow matmuls (hi@hi, lo@hi, hi@lo at 0.5 cyc/row each)
    replace two bf16 matmuls (1.0 cyc/row): 25% fewer PE cycles at
    measured max-rel 3.6e-3 (sim) vs the 2e-2 gate.
  - scales: x*8, w_attn*128, w_proj*128, all powers of 2 folded out
    exactly: qkv planes carry 1024x, exp scale /2^20, ones_col=128
    makes the normalized y carry 8x (prime fp8 range, residual clears
    subnormals), MM4 psum carries 1024x, host divides once.
  - normalize emits ytc hi (Pool copy) + lo (DVE sub) fp8 pair.
  - attention (scores/PV/denominator) stays bf16: full-fp8 there
    measured 1.7-4.5e-2 -- over or too near the gate.

Design (vs the fp32r baseline, 344.6us -> 259.3us):
  - bf16 end-to-end in SBUF/DRAM (fp32 only in PSUM accumulation):
    halves DMA bytes, removes the fp32r small-moving-dim 4x penalty.
    Validated numerics: rel err 4.3e-3 vs the 2e-2 gate.
  - chunky DMAs (~70 vs 405): the per-DMA ~650ns HWDGE/SP issue cost
    made the baseline sequencer-bound on DMA issue.
  - RoPE rotate-half without PE: the host permutes q/k head dims to
    even-first and folds the rotation sign into sin, so rotate-half is
    a partition half-swap done by two SBUF->SBUF DMAs issued right
    after each qkv eviction; all rope muls are cheap all-SBUF bf16 DVE
    ops (no J-matmul, no PSUM reads).
  - PSUM: 4-buffer "flex" pool (MM1 acc / transposes / scores /
    MM4 acc) + two 2-buffer banks that alternate between PV-accum and
    denominator roles each attention chunk, so a new chunk's first
    matmul lands on the earliest-released bank.
  - denominator: full tk-tiles pre-summed on DVE in groups of 8 (bf16
    2x mode), one accumulated ones-matmul per group; diagonal tiles
    accumulate directly on PE but are deferred one ti-step so the
    exp -> affine_select mask latency never stalls the PE.
  - attention cq0 runs interleaved inside phase A (MM1 fills its exp
    bubbles); each chunk's RoPE/v-transpose stage is pipelined into
    the next chunk's MM1; MM4 work flows through a global fill queue
    at ~1 item per ti-step, also absorbed by batch 1's phase A.
  - scores are software-pipelined one ti-step ahead; MM4 evictions
    split ~3/4 DVE / 1/4 ACT (ACT is the exp pacer).
  - PE p-state warmup during the initial DMA wait; ko-group-major
    first chunk starts compute on 1/8 of the first DMA.
"""

import math

import numpy as np

B, T, C = 2, 2048, 2048
NH = 16
NKV = 8
HD = 128
NCORES = 8
NQ = NH // NCORES   # 2 query heads per core
P = 128
TCH = 512           # phase-A token chunk
QCH = 512           # attention tq chunk (1 PSUM bank)
MMF = 512           # matmul moving free dim

_CACHE = {}


def _build_nc():
    import concourse.bacc as bacc
    import concourse.mybir as mybir
    from concourse.tile import TileContext
    from concourse.masks import make_identity

    F32 = mybir.dt.float32
    BF16 = mybir.dt.bfloat16
    F8 = mybir.dt.float8e4
    DR = mybir.MatmulPerfMode.DoubleRow
    AF = mybir.ActivationFunctionType
    ALU = mybir.AluOpType

    KO = C // P            # 16 contraction tiles for MM1
    KP = KO // 2           # 8 DoubleRow ko-pairs
    NT = NQ + 2            # 4 n-tiles (q0,q1,k,v)
    NL = NT * HD           # 512 local qkv cols
    TT = T // P            # 16 tk tiles per batch
    NCH = T // TCH         # 4 phase-A chunks per batch
    NQC = T // QCH         # 4 attention chunks per batch
    KTQ = QCH // P         # 4 tk-tiles per attention chunk step
    CCH = C // MMF         # 4 output column chunks
    GCN = B * NCH          # 8 global chunks
    scale = 1.0 / math.sqrt(HD)
    # x carries 8x, w_attn 128x -> raw qkv planes carry 1024x; cos/sin
    # carry 2^-10 so the fp8 q/k planes are true-scale; v keeps 1024x,
    # ones_col=128 makes the normalized y carry 8x; w_proj 128x -> out
    # carries 1024x (host divides once).
    EXPS = scale

    nc = bacc.Bacc("TRN2", target_bir_lowering=False)
    # {hi, lo} fp8 split on a middle dim (same scale per tensor); xt is
    # pre-chunked on the host so each chunk's (hi/lo, t) is contiguous
    # and the DMA balancer can merge the inner dims.
    xt = nc.dram_tensor("xt", [B * T // TCH, C, 2, TCH], F8,
                        kind="ExternalInput")
    wa = nc.dram_tensor("wa", [C, 2, NL], F8, kind="ExternalInput")
    wp = nc.dram_tensor("wp", [NQ * HD, 2, C], F8, kind="ExternalInput")
    cs = nc.dram_tensor("cs", [P, T], BF16, kind="ExternalInput")
    sn = nc.dram_tensor("sn", [P, T], BF16, kind="ExternalInput")
    out = nc.dram_tensor("out", [B * T, C], BF16, kind="ExternalOutput")

    with TileContext(nc) as tc:
        with tc.tile_pool(name="const", bufs=1) as constp, \
             tc.tile_pool(name="xtp", bufs=3) as xtp, \
             tc.tile_pool(name="plane", bufs=1) as planep, \
             tc.tile_pool(name="work", bufs=3) as workp, \
             tc.tile_pool(name="grpp", bufs=8) as grpp, \
             tc.tile_pool(name="small", bufs=2) as smallp, \
             tc.tile_pool(name="expp", bufs=6) as expp, \
             tc.tile_pool(name="ytcp", bufs=3) as ytcp, \
             tc.tile_pool(name="outp", bufs=4) as outp, \
             tc.tile_pool(name="ps", bufs=1, space="PSUM") as ps:

            # PSUM budget (8 banks): flex(4) + accA(2) + accB(2).
            # flex serves MM1 accumulators, rope jp, v transposes,
            # scores (sps) and MM4 accumulators; accA/accB alternate
            # between yacc and dacc roles per attention chunk so a new
            # chunk's yacc lands on the earliest-released banks.
            def mm_tile():
                return ps.tile([P, MMF], F32, tag="flex", name="flex",
                               bufs=4)

            sps_tile = mm_tile

            def acc_tile(par, nm):
                return ps.tile([P, QCH], F32, tag=f"acc{par}", name=nm,
                               bufs=2)

            cq_counter = [0]

            # PE warmup: dummy matmuls during the initial DMA wait keep
            # the tensor engine "continuously busy" so the p-state model
            # has it at full clock when real work arrives (~3.6us in).
            dummy = constp.tile([P, P], BF16, tag="dummy", name="dummy")
            nc.vector.memset(dummy, 0.0)
            for _ in range(36):
                nc.tensor.matmul(mm_tile()[:, :P], dummy, dummy,
                                 start=True, stop=True)

            # ---------------- constants + first chunks' DMA ------------
            wa_sb = constp.tile([P, KO, 2, NL], F8, tag="wa", name="wa")
            wa_r = wa.rearrange("(ko p) two n -> p ko two n", p=P)
            xt_r = xt.rearrange("g (ko p) two t -> g p ko two t", p=P)
            xt_tiles = {}
            xt0 = xtp.tile([P, KO, 2, TCH], F8, tag="xt", name="xt0")
            C0G = [(0, 2), (2, 4), (4, 6), (6, 8), (8, 10), (10, 12), (12, 16)]
            for (g0, g1) in C0G:
                nc.sync.dma_start(wa_sb[:, g0:g1], wa_r[:, g0:g1])
                nc.sync.dma_start(xt0[:, g0:g1], xt_r[0, :, g0:g1])
            xt_tiles[0] = xt0

            def emit_xt_dma(ci):
                tile = xtp.tile([P, KO, 2, TCH], F8, tag="xt",
                                name=f"xt{ci}")
                nc.sync.dma_start(tile[:, :KO // 2],
                                  xt_r[ci, :, :KO // 2])
                nc.sync.dma_start(tile[:, KO // 2:],
                                  xt_r[ci, :, KO // 2:])
                xt_tiles[ci] = tile

            def mm1_mms(acc, kp, n, xt_sb, start, stop, skip=False):
                """3 compensated DoubleRow matmuls for one ko-pair."""
                wh = wa_sb[:, 2 * kp:2 * kp + 2, 0, n * P:(n + 1) * P]
                wl = wa_sb[:, 2 * kp:2 * kp + 2, 1, n * P:(n + 1) * P]
                xh = xt_sb[:, 2 * kp:2 * kp + 2, 0]
                xl = xt_sb[:, 2 * kp:2 * kp + 2, 1]
                nc.tensor.matmul(acc, wh, xh, start=start, stop=False,
                                 perf_mode=DR, skip_group_check=skip)
                nc.tensor.matmul(acc, wl, xh, start=False, stop=False,
                                 perf_mode=DR, skip_group_check=skip)
                nc.tensor.matmul(acc, wh, xl, start=False, stop=stop,
                                 perf_mode=DR, skip_group_check=skip)

            cs_sb = constp.tile([P, T], BF16, tag="cs", name="cs")
            sn_sb = constp.tile([P, T], BF16, tag="sn", name="sn")
            nc.sync.dma_start(cs_sb[:, 0:TCH], cs[:, 0:TCH])
            nc.sync.dma_start(sn_sb[:, 0:TCH], sn[:, 0:TCH])
            emit_xt_dma(1)

            wp_sb = constp.tile([P, NQ, 2, C], F8, tag="wp", name="wp")
            wp_r = wp.rearrange("(ko p) two c -> p ko two c", p=P)

            ident_f = constp.tile([P, P], F32, tag="ident_f",
                                  name="ident_f")
            make_identity(nc, ident_f)
            ident = constp.tile([P, P], BF16, tag="ident", name="ident")
            nc.vector.tensor_copy(ident, ident_f)
            ones_f = constp.tile([P, 1], F32, tag="ones_f", name="ones_f")
            nc.vector.memset(ones_f, 128.0)  # folds v's 1024x into y=8x
            ones_col = constp.tile([P, 1], BF16, tag="ones_col",
                                   name="ones_col")
            nc.vector.tensor_copy(ones_col, ones_f)

            # ---------------- per-batch plane state ----------------
            q_planes = {}
            k_plane = {}
            v_nat = {}

            def alloc_planes(b):
                # q/k planes are fp8 in hd-split layout [64, 2, TCH]:
                # [:, 0] = rope dims 0..63, [:, 1] = dims 64..127, so
                # scores run as one DoubleRow matmul (2x64 contraction).
                # One tile PER CHUNK: the planes are DMA-written, and
                # per-chunk tiles keep the read deps exact.
                q_planes[b] = [[planep.tile([64, 2, TCH], F8,
                                            tag=f"qk{h}c{c}",
                                            name=f"q{h}b{b}c{c}", bufs=2)
                                for c in range(NCH)]
                               for h in range(NQ)]
                k_plane[b] = [planep.tile([64, 2, TCH], F8,
                                          tag=f"kplc{c}",
                                          name=f"kb{b}c{c}", bufs=2)
                              for c in range(NCH)]
                v_nat[b] = planep.tile([P, TT, HD], BF16, tag="vnat",
                                       name=f"vb{b}", bufs=2)

            # ---------------- MM4 (output projection) ----------------
            osb_hold = {}
            osb_mode = {}

            def emit_mm4(ytc_prev, row0, tl, cc, par,
                         half_dma=False):
                if cc == 0:
                    osb_hold[tl] = outp.tile([P, C], BF16, tag="o",
                                             name="o")
                    osb_mode[tl] = half_dma
                half_dma = osb_mode[tl]
                osb = osb_hold[tl]
                oacc = mm_tile()
                ytch, ytcl = ytc_prev
                ccs = slice(cc * MMF, (cc + 1) * MMF)
                yh = ytch[:, :, tl:tl + P]
                yl = ytcl[:, :, tl:tl + P]
                # lo-term last: gives the normalize sub (ytcl) the most
                # slack before the PE needs it
                nc.tensor.matmul(oacc, yh, wp_sb[:, :, 0, ccs],
                                 start=True, stop=False, perf_mode=DR)
                nc.tensor.matmul(oacc, yh, wp_sb[:, :, 1, ccs],
                                 start=False, stop=False, perf_mode=DR)
                nc.tensor.matmul(oacc, yl, wp_sb[:, :, 0, ccs],
                                 start=False, stop=True, perf_mode=DR)
                sl = osb[:, cc * MMF:(cc + 1) * MMF]
                # drain groups rotate evict engines per cc so the
                # tail's evictions pipeline across ACT+DVE+Pool;
                # mid-stream groups keep ACT at ~1/4 share (ACT is the
                # exp pacer)
                if half_dma:
                    eng = (nc.scalar.copy, nc.vector.tensor_copy,
                           nc.gpsimd.tensor_copy)[cc % 3]
                    eng(sl, oacc)
                elif par % 4 == 3:
                    nc.scalar.copy(sl, oacc)
                else:
                    nc.vector.tensor_copy(sl, oacc)
                # final-drain groups use half-tile DMAs (after cc1 and
                # cc3) so the first half transfers while the second
                # half computes; mid-stream groups use one full DMA.
                # The mode is latched per group at cc0 so a group that
                # straddles the drain boundary stays consistent.
                if half_dma == "cc":
                    nc.sync.dma_start(
                        out[row0:row0 + P, cc * MMF:(cc + 1) * MMF], sl)
                elif half_dma and cc in (1, CCH - 1):
                    h0c = 0 if cc == 1 else C // 2
                    nc.sync.dma_start(
                        out[row0:row0 + P, h0c:h0c + C // 2],
                        osb[:, h0c:h0c + C // 2])
                elif not half_dma and cc == CCH - 1:
                    nc.sync.dma_start(out[row0:row0 + P, :], osb)

            # global MM4 fill queue: attention pulls ~1-2 items per
            # ti-step; items flow across chunk boundaries
            fill_q = []

            def fill_push(ytc_prev, base):
                par = len(fill_q)
                for tl in range(0, QCH, P):
                    for cc in range(CCH):
                        fill_q.append((ytc_prev, base + tl, tl, cc,
                                       par))
                        par += 1

            def fill_pull(nmax):
                n = 0
                while fill_q and n < nmax:
                    emit_mm4(*fill_q.pop(0))
                    n += 1

            # ---------------- attention emitters ----------------
            class AttState:
                pass

            def att_begin(b, cq):
                st = AttState()
                st.b, st.cq = b, cq
                st.tq0 = cq * QCH
                st.ntk = KTQ * (cq + 1)
                par = cq_counter[0] % 2
                cq_counter[0] += 1
                st.yaccs = [acc_tile(par, f"yac{h}") for h in range(NQ)]
                st.daccs = [acc_tile(1 - par, f"dac{h}")[:1, :]
                            for h in range(NQ)]
                st.grps = [[None] * ((cq * KTQ + 7) // 8)
                           for _ in range(NQ)]
                st.exs = {}
                st.deferred = []
                st.ytch = ytcp.tile([P, NQ, QCH], F8, tag="ytch",
                                    name="ytch")
                st.ytcl = ytcp.tile([P, NQ, QCH], F8, tag="ytcl",
                                    name="ytcl")
                return st

            def att_drain_deferred(st):
                """Masked-diagonal matmuls, deferred one ti-step so the
                exp->affine_select latency never stalls the PE. Only
                start=False accumulations may be deferred: a start=True
                matmul resets its PSUM bank, so it must stay the first
                write (handled inline in att_fin)."""
                for (ti, h, ex, o, last, do_y) in st.deferred:
                    if do_y:
                        nc.tensor.matmul(
                            st.yaccs[h][:, o:o + P], v_nat[st.b][:, ti],
                            ex[:, o:o + P], start=False, stop=last,
                            skip_group_check=True)
                    nc.tensor.matmul(
                        st.daccs[h][:, o:], ones_col, ex[:, o:],
                        start=(ti == KTQ * st.cq),
                        stop=(st.cq == 0 and last),
                        skip_group_check=True)
                st.deferred = []

            def att_sps(st, ti, h):
                o = max(0, (ti - KTQ * st.cq) * P)
                sps = sps_tile()
                kc = (ti % KTQ) * P
                nc.tensor.matmul(
                    sps[:, o:],
                    k_plane[st.b][ti // KTQ][:, :, kc:kc + P],
                    q_planes[st.b][h][st.cq][:, :, o:QCH],
                    start=True, stop=True, perf_mode=DR)
                ex = expp.tile([P, QCH], BF16, tag="exp", name="exp")
                nc.scalar.activation(ex[:, o:], sps[:, o:], AF.Exp,
                                     scale=EXPS)
                if ti >= KTQ * st.cq:  # diagonal tile: mask tq < tk
                    nc.gpsimd.affine_select(
                        ex[:, o:o + P], ex[:, o:o + P],
                        pattern=[[1, P]], compare_op=ALU.is_ge,
                        fill=0.0, base=0, channel_multiplier=-1)
                st.exs[(ti, h)] = ex

            def att_fin(st, ti, h):
                o = max(0, (ti - KTQ * st.cq) * P)
                first, last = (ti == 0), (ti == st.ntk - 1)
                ex = st.exs.pop((ti, h))
                if ti < KTQ * st.cq:
                    nc.tensor.matmul(
                        st.yaccs[h][:, o:], v_nat[st.b][:, ti],
                        ex[:, o:], start=first, stop=last,
                        skip_group_check=True)
                    # full tile: accumulate into its group-of-8 sum
                    eng = nc.vector
                    gi = ti // 8
                    if ti % 8 == 0:
                        g = grpp.tile([P, QCH], BF16, tag="grp",
                                      name="grp")
                        eng.tensor_copy(g, ex)
                        st.grps[h][gi] = g
                    else:
                        g = st.grps[h][gi]
                        eng.tensor_add(g, g, ex)
                elif first:
                    # cq0's first tile: single start=True write for the
                    # whole bank (start resets the bank, so it cannot be
                    # split); waits for the affine mask, but the fine
                    # interleave absorbs that
                    nc.tensor.matmul(
                        st.yaccs[h][:, o:], v_nat[st.b][:, ti],
                        ex[:, o:], start=True, stop=False,
                        skip_group_check=True)
                    st.deferred.append((ti, h, ex, o, last, False))
                else:
                    # diagonal tile: the unmasked suffix can run now;
                    # the masked 128-wide block + denominator defer one
                    # ti-step (see att_drain_deferred)
                    if o + P < QCH:
                        nc.tensor.matmul(
                            st.yaccs[h][:, o + P:],
                            v_nat[st.b][:, ti], ex[:, o + P:],
                            start=False, stop=False,
                            skip_group_check=True)
                    st.deferred.append((ti, h, ex, o, last, True))
                if last:
                    att_drain_deferred(st)
                    # close the denominator (group matmuls) and start
                    # this head's normalize chain immediately
                    for gi in range(len(st.grps[h])):
                        nc.tensor.matmul(
                            st.daccs[h], ones_col, st.grps[h][gi],
                            start=False,
                            stop=(gi == len(st.grps[h]) - 1),
                            skip_group_check=True)
                    rec_f = smallp.tile([1, QCH], F32, tag="rec_f",
                                        name="rec_f")
                    nc.vector.reciprocal(rec_f, st.daccs[h])
                    bcs = workp.tile([P, QCH], F32, tag="bcast",
                                     name="bcast")
                    nc.gpsimd.partition_broadcast(bcs, rec_f)
                    # m = 8*y (fp32) -> hi (Pool) + lo (DVE) fp8 pair,
                    # in column halves so the first MM4 row-groups of
                    # this chunk unblock after half the chain
                    mn = workp.tile([P, QCH], F32, tag="mnorm",
                                    name="mnorm")
                    for hf in range(2):
                        csl = slice(hf * (QCH // 2), (hf + 1) * (QCH // 2))
                        nc.vector.tensor_mul(mn[:, csl],
                                             st.yaccs[h][:, csl],
                                             bcs[:, csl])
                        nc.gpsimd.tensor_copy(st.ytch[:, h, csl],
                                              mn[:, csl])
                        nc.vector.tensor_sub(st.ytcl[:, h, csl],
                                             mn[:, csl],
                                             st.ytch[:, h, csl])

            def attention_steps(b, cq, fills=0, prelude=None,
                                fine=False):
                """Yields after each sub-phase; last yield is the state
                (with .ytc set). fine=True yields after every single
                emission (for interleaving into phase A). With fills,
                pulls `fills` items/ti from the global queue, and scores
                are software-pipelined one ti-step ahead."""
                st = att_begin(b, cq)
                if not fine:
                    att_sps(st, 0, 0)
                    att_sps(st, 0, 1)
                    yield None
                for ti in range(st.ntk):
                    if fine:
                        att_sps(st, ti, 0)
                        yield None
                        att_sps(st, ti, 1)
                        yield None
                    if prelude is not None:
                        next(prelude, None)
                    if fills:
                        # ti=0 pulls would hit items whose normalize
                        # chain just started; defer them to the tail
                        if ti == 0:
                            pass
                        elif ti >= st.ntk - 2:
                            fill_pull(fills + 1)
                        else:
                            fill_pull(fills)
                    if not fine and ti + 1 < st.ntk:
                        att_sps(st, ti + 1, 0)
                        att_sps(st, ti + 1, 1)
                    if ti < st.ntk - 1:
                        att_drain_deferred(st)
                    att_fin(st, ti, 0)
                    if fine:
                        yield None
                    att_fin(st, ti, 1)
                    yield None
                yield st

            # ---------------- phase A ----------------
            def phase_a_prologue(b, c):
                gc = b * NCH + c
                if b == 0 and c + 1 < NCH:
                    t1 = (c + 1) * TCH
                    nc.sync.dma_start(cs_sb[:, t1:t1 + TCH],
                                      cs[:, t1:t1 + TCH])
                    nc.sync.dma_start(sn_sb[:, t1:t1 + TCH],
                                      sn[:, t1:t1 + TCH])
                if b == 0 and c == NCH - 1:
                    for ko in range(NQ):
                        nc.sync.dma_start(wp_sb[:, ko], wp_r[:, ko])

            def mm1_steps(b, c, first_chunk=False):
                """Generator: per n-step, the 16-ko MM1 chain + evict."""
                phase_a_prologue(b, c)
                gc = b * NCH + c
                xt_sb = xt_tiles.pop(gc)
                raw2 = workp.tile([P, NQ, TCH], BF16, tag="raw2",
                                  name="raw2")
                rawk = workp.tile([P, TCH], BF16, tag="rawk", name="rawk")
                vTc = workp.tile([P, TCH], BF16, tag="vTc", name="vTc")
                tmps = {}

                def evict(n, acc):
                    if n < NQ:
                        nc.scalar.copy(raw2[:, n], acc)
                        src_ap = raw2
                        tmps[n] = workp.tile([P, TCH], BF16, tag="tmp",
                                             name="tmp", bufs=6)
                        nc.sync.dma_start(tmps[n][:P // 2],
                                          raw2[P // 2:, n])
                        nc.sync.dma_start(tmps[n][P // 2:],
                                          raw2[:P // 2, n])
                    elif n == NQ:
                        nc.scalar.copy(rawk, acc)
                        tmps[n] = workp.tile([P, TCH], BF16, tag="tmp",
                                             name="tmp", bufs=6)
                        nc.sync.dma_start(tmps[n][:P // 2],
                                          rawk[P // 2:])
                        nc.sync.dma_start(tmps[n][P // 2:],
                                          rawk[:P // 2])
                    else:
                        # v evict on Pool: keeps ACT free for exp and
                        # decouples the v-transpose from the exp stream
                        nc.gpsimd.tensor_copy(vTc, acc)

                if first_chunk:
                    # ko-group-major over the 4 flex banks: PE starts
                    # after the first 2-ko slice of the wa/xt DMA.
                    accs = [mm_tile() for _ in range(NT)]
                    for (g0, g1) in C0G:
                        for n in range(NT):
                            for kp in range(g0 // 2, g1 // 2):
                                mm1_mms(accs[n], kp, n, xt_sb,
                                        start=(kp == 0),
                                        stop=(kp == KP - 1), skip=True)
                        yield None
                    for n in range(NT):
                        evict(n, accs[n])
                else:
                    for n in range(NT):
                        acc = mm_tile()
                        for kp in range(KP):
                            mm1_mms(acc, kp, n, xt_sb,
                                    start=(kp == 0),
                                    stop=(kp == KP - 1))
                        evict(n, acc)
                        yield None
                # xt prefetch issued AFTER this chunk's evict/tmp DMAs:
                # keeps the small latency-critical transfers (tmps, fp8
                # planes) ahead of the next 2MB stream in the queues
                if gc + 2 < GCN:
                    emit_xt_dma(gc + 2)
                yield (raw2, rawk, vTc, tmps)

            def rope_steps(b, c, raw2, rawk, vTc, tmps):
                """Generator: 4 steps: rope(q0), rope(q1), rope(k),
                v transposes. The half-swapped tmps were produced by
                DMA right after each eviction in mm1_steps."""
                t0 = c * TCH

                def rope_one(src, tmp, dst):
                    t2 = workp.tile([P, TCH], BF16, tag="t2", name="t2")
                    nc.vector.tensor_mul(t2, tmp, sn_sb[:, t0:t0 + TCH])
                    t1 = workp.tile([P, TCH], BF16, tag="t1", name="t1")
                    nc.vector.tensor_mul(t1, src, cs_sb[:, t0:t0 + TCH])
                    tb = workp.tile([P, TCH], BF16, tag="tb", name="tb",
                                    bufs=4)
                    nc.vector.tensor_add(tb, t1, t2)
                    # fp8 convert (Pool) + partition half-swap into the
                    # hd-split [64, 2, T] plane (2 SBUF->SBUF DMAs)
                    f8p = workp.tile([P, TCH], F8, tag="f8p", name="f8p",
                                     bufs=4)
                    nc.gpsimd.tensor_copy(f8p, tb)
                    nc.sync.dma_start(dst[:, 0, :], f8p[:P // 2])
                    nc.sync.dma_start(dst[:, 1, :], f8p[P // 2:])

                for h in range(NQ):
                    rope_one(raw2[:, h], tmps[h], q_planes[b][h][c])
                    yield None
                rope_one(rawk, tmps[NQ], k_plane[b][c])
                yield None
                for i in range(KTQ):
                    pt = mm_tile().bitcast(BF16)[:, :P]
                    nc.tensor.transpose(
                        pt, vTc[:, i * P:(i + 1) * P], ident)
                    nc.vector.tensor_copy(v_nat[b][:, t0 // P + i], pt)
                    if i % 2 == 1:
                        yield None

            def phase_a_batch(b, start_slot, pull_aux, fills=False):
                """Run phase A of batch b with chunk-pipelined rope and
                aux pulls (2 per slot from start_slot on). Returns the
                final chunk's rope generator (not drained)."""
                slot = [0]

                def slot_tick():
                    slot[0] += 1
                    if fills:
                        fill_pull(1)
                    if slot[0] >= start_slot:
                        pull_aux()
                        pull_aux()

                prev_rope = None
                for c in range(NCH):
                    mm1 = mm1_steps(b, c, first_chunk=(b == 0 and c == 0))
                    tail = None
                    for v in mm1:
                        if v is not None:
                            tail = v
                            break
                        if prev_rope is not None:
                            next(prev_rope, None)
                        slot_tick()
                    if prev_rope is not None:
                        for _ in prev_rope:
                            slot_tick()
                    prev_rope = rope_steps(b, c, *tail)
                return prev_rope

            # ================= emission schedule =================
            att_cq0 = {}
            att_cq0_done = {}

            def make_cq0_puller(b):
                gen = attention_steps(b, 0, fine=True)
                att_cq0[b] = gen

                def pull():
                    v = next(gen, False)
                    if v is not False and v is not None:
                        att_cq0_done[b] = v
                return pull

            def drain_cq0(b, rope_tail):
                while b not in att_cq0_done:
                    v = next(att_cq0[b], False)
                    if v is False:
                        break
                    if v is not None:
                        att_cq0_done[b] = v
                    if rope_tail is not None:
                        next(rope_tail, None)

            def chain(*its):
                for it in its:
                    yield from it

            def run_attention(b, cq, prelude=None, fills=1):
                st = None
                for v in attention_steps(b, cq, fills=fills,
                                         prelude=prelude):
                    if v is not None:
                        st = v
                fill_push((st.ytch, st.ytcl), b * T + cq * QCH)

            # ---- batch 0 ----
            alloc_planes(0)
            pull0 = make_cq0_puller(0)
            rope_tail = phase_a_batch(0, 14, pull0)
            drain_cq0(0, rope_tail)
            fill_push((att_cq0_done[0].ytch, att_cq0_done[0].ytcl), 0)

            run_attention(0, 1, prelude=rope_tail)
            run_attention(0, 2)
            run_attention(0, 3)

            # ---- batch 1 ----
            alloc_planes(1)
            pull1 = make_cq0_puller(1)
            rope_tail = phase_a_batch(1, 10, pull1, fills=True)
            drain_cq0(1, rope_tail)
            fill_push((att_cq0_done[1].ytch, att_cq0_done[1].ytcl), T)

            run_attention(1, 1, prelude=rope_tail)
            run_attention(1, 2)
            run_attention(1, 3)
            while fill_q:
                emit_mm4(*fill_q.pop(0), half_dma=True)

    nc.finalize()
    return nc


def _host_prep(x, w_attn, w_proj, freqs_cos, freqs_sin):
    """Shard + relayout inputs for the 8 cores (head-parallel).

    x/w_attn/w_proj ship as fp8e4 hi+lo pairs (same pow2 scale: hi =
    fp8(a*s), lo = fp8(a*s - hi)), stacked on dim1: [rows, 2, cols]."""
    import ml_dtypes
    BF = ml_dtypes.bfloat16
    F8 = ml_dtypes.float8_e4m3

    def split8(a, sc):
        a = np.ascontiguousarray(a * sc)
        hi = a.astype(F8)
        lo = (a - hi.astype(np.float32)).astype(F8)
        return np.ascontiguousarray(np.stack([hi, lo], axis=1))

    x = np.asarray(x, dtype=np.float32)
    w_attn = np.asarray(w_attn, dtype=np.float32)
    w_proj = np.asarray(w_proj, dtype=np.float32)
    fc = np.asarray(freqs_cos, dtype=np.float32)
    fs = np.asarray(freqs_sin, dtype=np.float32)

    # [C, 2, B*T] fp8 -> pre-chunked [B*T/TCH, C, 2, TCH]
    xt = split8(x.reshape(B * T, C).T, 8.0)
    xt = np.ascontiguousarray(
        xt.reshape(C, 2, B * T // TCH, TCH).transpose(2, 0, 1, 3))
    # head-dim layout for q/k is permuted to even-dims-first so that
    # rotate-half becomes a partition half-swap on device; the rotation
    # sign is folded into the sin tensor (top half negated)
    perm = np.concatenate([np.arange(0, HD, 2), np.arange(1, HD, 2)])
    cs_i = np.repeat(fc, 2, axis=1).T  # [HD, T] interleaved layout
    sn_i = np.repeat(fs, 2, axis=1).T
    # cos/sin carry 2^-10 to descale the 1024x raw q/k during rope
    cs = np.ascontiguousarray(cs_i[perm] / 1024.0).astype(BF)
    sgn = np.where(np.arange(HD) < HD // 2, -1.0, 1.0)[:, None]
    sn = np.ascontiguousarray(sn_i[perm] * sgn / 1024.0).astype(
        np.float32).astype(BF)

    in_maps = []
    for g in range(NCORES):
        q0 = w_attn[:, 2 * g * HD:(2 * g + 1) * HD][:, perm]
        q1 = w_attn[:, (2 * g + 1) * HD:(2 * g + 2) * HD][:, perm]
        k_cols = w_attn[:, NH * HD + g * HD:
                        NH * HD + (g + 1) * HD][:, perm]
        v_cols = w_attn[:, (NH + NKV) * HD + g * HD:
                        (NH + NKV) * HD + (g + 1) * HD]
        wa_g = split8(
            np.concatenate([q0, q1, k_cols, v_cols], axis=1), 128.0)
        wp_g = split8(w_proj[2 * g * HD:(2 * g + 2) * HD, :], 128.0)
        in_maps.append({"xt": xt, "wa": wa_g, "wp": wp_g,
                        "cs": cs, "sn": sn})
    return in_maps


def kernel(x, w_attn, w_proj, freqs_cos, freqs_sin):
    from concourse.bass_utils import run_bass_kernel_spmd

    if "nc" not in _CACHE:
        _CACHE["nc"] = _build_nc()
    nc = _CACHE["nc"]
    in_maps = _host_prep(x, w_attn, w_proj, freqs_cos, freqs_sin)
    res = run_bass_kernel_spmd(nc, in_maps, core_ids=list(range(NCORES)))
    acc = np.zeros((B * T, C), dtype=np.float64)
    for r in res.results:
        acc += np.asarray(r["out"], dtype=np.float64)
    acc *= 1.0 / 1024.0  # fold out the 8x (y) * 128x (w_proj) scales
    return acc.reshape(B, T, C).astype(np.float32)



# revision 49
# speedup vs baseline: 1.0349x; 1.0327x over previous
"""Trainium2 Bass kernel for CausalSelfAttention (GQA + RoPE).

Sharding: tensor-parallel over heads across 8 cores (2 q heads + 1 kv
head per core); host sums the 8 partial output projections.

fp8 DoubleRow (vs the bf16 kernel, 259.3us -> target ~205us):
  - MM1 and MM4 run as compensated fp8e4 DoubleRow matmuls: each
    operand is split hi+lo (same power-of-2 scale; lo = fp8(a - hi),
    so hi+lo carries ~0.2% error, bf16-grade). Per ko-PAIR of 128,
    three DoubleRow matmuls (hi@hi, lo@hi, hi@lo at 0.5 cyc/row each)
    replace two bf16 matmuls (1.0 cyc/row): 25% fewer PE cycles at
    measured max-rel 3.6e-3 (sim) vs the 2e-2 gate.
  - scales: x*8, w_attn*128, w_proj*128, all powers of 2 folded out
    exactly: qkv planes carry 1024x, exp scale /2^20, ones_col=128
    makes the normalized y carry 8x (prime fp8 range, residual clears
    subnormals), MM4 psum carries 1024x, host divides once.
  - normalize emits ytc hi (Pool copy) + lo (DVE sub) fp8 pair.
  - attention (scores/PV/denominator) stays bf16: full-fp8 there
    measured 1.7-4.5e-2 -- over or too near the gate.

Design (vs the fp32r baseline, 344.6us -> 259.3us):
  - bf16 end-to-end in SBUF/DRAM (fp32 only in PSUM accumulation):
    halves DMA bytes, removes the fp32r small-moving-dim 4x penalty.
    Validated numerics: rel err 4.3e-3 vs the 2e-2 gate.
  - chunky DMAs (~70 vs 405): the per-DMA ~650ns HWDGE/SP issue cost
    made the baseline sequencer-bound on DMA issue.
  - RoPE rotate-half without PE: the host permutes q/k head dims to
    even-first and folds the rotation sign into sin, so rotate-half is
    a partition half-swap done by two SBUF->SBUF DMAs issued right
    after each qkv eviction; all rope muls are cheap all-SBUF bf16 DVE
    ops (no J-matmul, no PSUM reads).
  - PSUM: 4-buffer "flex" pool (MM1 acc / transposes / scores /
    MM4 acc) + two 2-buffer banks that alternate between PV-accum and
    denominator roles each attention chunk, so a new chunk's first
    matmul lands on the earliest-released bank.
  - denominator: full tk-tiles pre-summed on DVE in groups of 8 (bf16
    2x mode), one accumulated ones-matmul per group; diagonal tiles
    accumulate directly on PE but are deferred one ti-step so the
    exp -> affine_select mask latency never stalls the PE.
  - attention cq0 runs interleaved inside phase A (MM1 fills its exp
    bubbles); each chunk's RoPE/v-transpose stage is pipelined into
    the next chunk's MM1; MM4 work flows through a global fill queue
    at ~1 item per ti-step, also absorbed by batch 1's phase A.
  - scores are software-pipelined one ti-step ahead; MM4 evictions
    split ~3/4 DVE / 1/4 ACT (ACT is the exp pacer).
  - PE p-state warmup during the initial DMA wait; ko-group-major
    first chunk starts compute on 1/8 of the first DMA.
"""

import math

import numpy as np

B, T, C = 2, 2048, 2048
NH = 16
NKV = 8
HD = 128
NCORES = 8
NQ = NH // NCORES   # 2 query heads per core
P = 128
TCH = 512           # phase-A token chunk
QCH = 512           # attention tq chunk (1 PSUM bank)
MMF = 512           # matmul moving free dim

_CACHE = {}


def _build_nc():
    import concourse.bacc as bacc
    import concourse.mybir as mybir
    from concourse.tile import TileContext
    from concourse.masks import make_identity

    F32 = mybir.dt.float32
    BF16 = mybir.dt.bfloat16
    F8 = mybir.dt.float8e4
    DR = mybir.MatmulPerfMode.DoubleRow
    AF = mybir.ActivationFunctionType
    ALU = mybir.AluOpType

    KO = C // P            # 16 contraction tiles for MM1
    KP = KO // 2           # 8 DoubleRow ko-pairs
    NT = NQ + 2            # 4 n-tiles (q0,q1,k,v)
    NL = NT * HD           # 512 local qkv cols
    TT = T // P            # 16 tk tiles per batch
    NCH = T // TCH         # 4 phase-A chunks per batch
    NQC = T // QCH         # 4 attention chunks per batch
    KTQ = QCH // P         # 4 tk-tiles per attention chunk step
    CCH = C // MMF         # 4 output column chunks
    GCN = B * NCH          # 8 global chunks
    scale = 1.0 / math.sqrt(HD)
    # x carries 8x, w_attn 128x -> raw qkv planes carry 1024x; cos/sin
    # carry 2^-10 so the fp8 q/k planes are true-scale; v keeps 1024x,
    # ones_col=128 makes the normalized y carry 8x; w_proj 128x -> out
    # carries 1024x (host divides once).
    EXPS = scale

    nc = bacc.Bacc("TRN2", target_bir_lowering=False)
    # {hi, lo} fp8 split on a middle dim (same scale per tensor); xt is
    # pre-chunked on the host so each chunk's (hi/lo, t) is contiguous
    # and the DMA balancer can merge the inner dims.
    xt = nc.dram_tensor("xt", [B * T // TCH, C, 2, TCH], F8,
                        kind="ExternalInput")
    wa = nc.dram_tensor("wa", [C, 2, NL], F8, kind="ExternalInput")
    wp = nc.dram_tensor("wp", [NQ * HD, 2, C], F8, kind="ExternalInput")
    cs = nc.dram_tensor("cs", [P, T], BF16, kind="ExternalInput")
    sn = nc.dram_tensor("sn", [P, T], BF16, kind="ExternalInput")
    out = nc.dram_tensor("out", [B * T, C], BF16, kind="ExternalOutput")

    with TileContext(nc) as tc:
        with tc.tile_pool(name="const", bufs=1) as constp, \
             tc.tile_pool(name="xtp", bufs=3) as xtp, \
             tc.tile_pool(name="plane", bufs=1) as planep, \
             tc.tile_pool(name="work", bufs=3) as workp, \
             tc.tile_pool(name="grpp", bufs=8) as grpp, \
             tc.tile_pool(name="small", bufs=2) as smallp, \
             tc.tile_pool(name="expp", bufs=6) as expp, \
             tc.tile_pool(name="ytcp", bufs=3) as ytcp, \
             tc.tile_pool(name="outp", bufs=4) as outp, \
             tc.tile_pool(name="ps", bufs=1, space="PSUM") as ps:

            # PSUM budget (8 banks): flex(5) + acc(3-of-4-buf pool).
            # flex serves MM1 accumulators, rope jp, v transposes,
            # scores (sps) and MM4 accumulators. The acc pool rotates
            # yac0,yac1,dac through 4 banks (3 allocs/chunk), so a new
            # chunk's yaccs land on the earliest-released banks (the
            # previous dac, freed right after the reciprocal). Both
            # heads' [1,512] denominators share ONE bank at partitions
            # 0 and 32 (tile_position requires 0/32/64/96).
            def mm_tile():
                return ps.tile([P, MMF], F32, tag="flex", name="flex",
                               bufs=5)

            sps_tile = mm_tile

            def acc_tile(nm):
                return ps.tile([P, QCH], F32, tag="acc", name=nm,
                               bufs=3)

            cq_counter = [0]

            # PE warmup: dummy matmuls during the initial DMA wait keep
            # the tensor engine "continuously busy" so the p-state model
            # has it at full clock when real work arrives (~3.6us in).
            dummy = constp.tile([P, P], BF16, tag="dummy", name="dummy")
            nc.vector.memset(dummy, 0.0)
            for _ in range(36):
                nc.tensor.matmul(mm_tile()[:, :P], dummy, dummy,
                                 start=True, stop=True)

            # ---------------- constants + first chunks' DMA ------------
            wa_sb = constp.tile([P, KO, 2, NL], F8, tag="wa", name="wa")
            wa_r = wa.rearrange("(ko p) two n -> p ko two n", p=P)
            xt_r = xt.rearrange("g (ko p) two t -> g p ko two t", p=P)
            xt_tiles = {}
            xt0 = xtp.tile([P, KO, 2, TCH], F8, tag="xt", name="xt0")
            C0G = [(0, 2), (2, 4), (4, 6), (6, 8), (8, 10), (10, 12), (12, 16)]
            for (g0, g1) in C0G:
                nc.sync.dma_start(wa_sb[:, g0:g1], wa_r[:, g0:g1])
                nc.sync.dma_start(xt0[:, g0:g1], xt_r[0, :, g0:g1])
            xt_tiles[0] = xt0

            def emit_xt_dma(ci):
                tile = xtp.tile([P, KO, 2, TCH], F8, tag="xt",
                                name=f"xt{ci}")
                nc.sync.dma_start(tile[:, :KO // 2],
                                  xt_r[ci, :, :KO // 2])
                nc.sync.dma_start(tile[:, KO // 2:],
                                  xt_r[ci, :, KO // 2:])
                xt_tiles[ci] = tile

            def mm1_mms(acc, kp, n, xt_sb, start, stop, skip=False):
                """3 compensated DoubleRow matmuls for one ko-pair."""
                wh = wa_sb[:, 2 * kp:2 * kp + 2, 0, n * P:(n + 1) * P]
                wl = wa_sb[:, 2 * kp:2 * kp + 2, 1, n * P:(n + 1) * P]
                xh = xt_sb[:, 2 * kp:2 * kp + 2, 0]
                xl = xt_sb[:, 2 * kp:2 * kp + 2, 1]
                nc.tensor.matmul(acc, wh, xh, start=start, stop=False,
                                 perf_mode=DR, skip_group_check=skip)
                nc.tensor.matmul(acc, wl, xh, start=False, stop=False,
                                 perf_mode=DR, skip_group_check=skip)
                nc.tensor.matmul(acc, wh, xl, start=False, stop=stop,
                                 perf_mode=DR, skip_group_check=skip)

            cs_sb = constp.tile([P, T], BF16, tag="cs", name="cs")
            sn_sb = constp.tile([P, T], BF16, tag="sn", name="sn")
            nc.sync.dma_start(cs_sb[:, 0:TCH], cs[:, 0:TCH])
            nc.sync.dma_start(sn_sb[:, 0:TCH], sn[:, 0:TCH])
            emit_xt_dma(1)

            wp_sb = constp.tile([P, NQ, 2, C], F8, tag="wp", name="wp")
            wp_r = wp.rearrange("(ko p) two c -> p ko two c", p=P)

            ident_f = constp.tile([P, P], F32, tag="ident_f",
                                  name="ident_f")
            make_identity(nc, ident_f)
            ident = constp.tile([P, P], BF16, tag="ident", name="ident")
            nc.vector.tensor_copy(ident, ident_f)
            ones_f = constp.tile([P, 1], F32, tag="ones_f", name="ones_f")
            nc.vector.memset(ones_f, 128.0)  # folds v's 1024x into y=8x
            ones_col = constp.tile([P, 1], BF16, tag="ones_col",
                                   name="ones_col")
            nc.vector.tensor_copy(ones_col, ones_f)

            # ---------------- per-batch plane state ----------------
            q_planes = {}
            k_plane = {}
            v_nat = {}

            def alloc_planes(b):
                # q/k planes are fp8 in hd-split layout [64, 2, TCH]:
                # [:, 0] = rope dims 0..63, [:, 1] = dims 64..127, so
                # scores run as one DoubleRow matmul (2x64 contraction).
                # One tile PER CHUNK: the planes are DMA-written, and
                # per-chunk tiles keep the read deps exact.
                q_planes[b] = [[planep.tile([64, 2, TCH], F8,
                                            tag=f"qk{h}c{c}",
                                            name=f"q{h}b{b}c{c}", bufs=2)
                                for c in range(NCH)]
                               for h in range(NQ)]
                k_plane[b] = [planep.tile([64, 2, TCH], F8,
                                          tag=f"kplc{c}",
                                          name=f"kb{b}c{c}", bufs=2)
                              for c in range(NCH)]
                v_nat[b] = planep.tile([P, TT, HD], BF16, tag="vnat",
                                       name=f"vb{b}", bufs=2)

            # ---------------- MM4 (output projection) ----------------
            osb_hold = {}
            osb_mode = {}

            def emit_mm4(ytc_prev, row0, tl, cc, par,
                         half_dma=False):
                if cc == 0:
                    osb_hold[tl] = outp.tile([P, C], BF16, tag="o",
                                             name="o")
                    osb_mode[tl] = half_dma
                half_dma = osb_mode[tl]
                osb = osb_hold[tl]
                oacc = mm_tile()
                ytch, ytcl = ytc_prev
                ccs = slice(cc * MMF, (cc + 1) * MMF)
                yh = ytch[:, :, tl:tl + P]
                yl = ytcl[:, :, tl:tl + P]
                # lo-term last: gives the normalize sub (ytcl) the most
                # slack before the PE needs it
                nc.tensor.matmul(oacc, yh, wp_sb[:, :, 0, ccs],
                                 start=True, stop=False, perf_mode=DR)
                nc.tensor.matmul(oacc, yh, wp_sb[:, :, 1, ccs],
                                 start=False, stop=False, perf_mode=DR)
                nc.tensor.matmul(oacc, yl, wp_sb[:, :, 0, ccs],
                                 start=False, stop=True, perf_mode=DR)
                sl = osb[:, cc * MMF:(cc + 1) * MMF]
                # drain groups alternate evict engines per cc so the
                # tail's 4 evictions pipeline across ACT+DVE (Pool
                # cannot read PSUM); mid-stream groups keep ACT at
                # ~1/4 share (ACT is the exp pacer)
                evict_act = (cc % 2 == 1) if half_dma else (
                    par % 4 == 3)
                if evict_act:
                    nc.scalar.copy(sl, oacc)
                else:
                    nc.vector.tensor_copy(sl, oacc)
                # final-drain groups use half-tile DMAs (after cc1 and
                # cc3) so the first half transfers while the second
                # half computes; mid-stream groups use one full DMA.
                # The mode is latched per group at cc0 so a group that
                # straddles the drain boundary stays consistent.
                if half_dma == "cc":
                    nc.sync.dma_start(
                        out[row0:row0 + P, cc * MMF:(cc + 1) * MMF], sl)
                elif half_dma and cc in (1, CCH - 1):
                    h0c = 0 if cc == 1 else C // 2
                    nc.sync.dma_start(
                        out[row0:row0 + P, h0c:h0c + C // 2],
                        osb[:, h0c:h0c + C // 2])
                elif not half_dma and cc == CCH - 1:
                    nc.sync.dma_start(out[row0:row0 + P, :], osb)

            # global MM4 fill queue: attention pulls ~1-2 items per
            # ti-step; items flow across chunk boundaries
            fill_q = []

            def fill_push(ytc_prev, base):
                par = len(fill_q)
                for tl in range(0, QCH, P):
                    for cc in range(CCH):
                        fill_q.append((ytc_prev, base + tl, tl, cc,
                                       par))
                        par += 1

            def fill_pull(nmax):
                n = 0
                while fill_q and n < nmax:
                    emit_mm4(*fill_q.pop(0))
                    n += 1

            # ---------------- attention emitters ----------------
            class AttState:
                pass

            def att_begin(b, cq):
                st = AttState()
                st.b, st.cq = b, cq
                st.tq0 = cq * QCH
                st.ntk = KTQ * (cq + 1)
                st.yaccs = [acc_tile(f"yac{h}") for h in range(NQ)]
                dtile = acc_tile("dac")
                st.daccs = [dtile[32 * h:32 * h + 1, :]
                            for h in range(NQ)]
                st.grps = [[None] * ((cq * KTQ + 7) // 8)
                           for _ in range(NQ)]
                st.exs = {}
                st.deferred = []
                st.ytch = ytcp.tile([P, NQ, QCH], F8, tag="ytch",
                                    name="ytch")
                st.ytcl = ytcp.tile([P, NQ, QCH], F8, tag="ytcl",
                                    name="ytcl")
                return st

            def att_drain_deferred(st):
                """Masked-diagonal matmuls, deferred one ti-step so the
                exp->affine_select latency never stalls the PE. Only
                start=False accumulations may be deferred: a start=True
                matmul resets its PSUM bank, so it must stay the first
                write (handled inline in att_fin)."""
                for (ti, h, ex, o, last, do_y) in st.deferred:
                    if do_y:
                        nc.tensor.matmul(
                            st.yaccs[h][:, o:o + P], v_nat[st.b][:, ti],
                            ex[:, o:o + P], start=False, stop=last,
                            skip_group_check=True)
                    nc.tensor.matmul(
                        st.daccs[h][:, o:], ones_col, ex[:, o:],
                        start=(ti == KTQ * st.cq),
                        stop=(st.cq == 0 and last),
                        skip_group_check=True)
                st.deferred = []

            def att_sps(st, ti, h):
                o = max(0, (ti - KTQ * st.cq) * P)
                sps = sps_tile()
                kc = (ti % KTQ) * P
                nc.tensor.matmul(
                    sps[:, o:],
                    k_plane[st.b][ti // KTQ][:, :, kc:kc + P],
                    q_planes[st.b][h][st.cq][:, :, o:QCH],
                    start=True, stop=True, perf_mode=DR)
                ex = expp.tile([P, QCH], BF16, tag="exp", name="exp")
                nc.scalar.activation(ex[:, o:], sps[:, o:], AF.Exp,
                                     scale=EXPS)
                if ti >= KTQ * st.cq:  # diagonal tile: mask tq < tk
                    nc.gpsimd.affine_select(
                        ex[:, o:o + P], ex[:, o:o + P],
                        pattern=[[1, P]], compare_op=ALU.is_ge,
                        fill=0.0, base=0, channel_multiplier=-1)
                st.exs[(ti, h)] = ex

            def att_fin(st, ti, h):
                o = max(0, (ti - KTQ * st.cq) * P)
                first, last = (ti == 0), (ti == st.ntk - 1)
                ex = st.exs.pop((ti, h))
                if ti < KTQ * st.cq:
                    nc.tensor.matmul(
                        st.yaccs[h][:, o:], v_nat[st.b][:, ti],
                        ex[:, o:], start=first, stop=last,
                        skip_group_check=True)
                    # full tile: accumulate into its group-of-8 sum
                    eng = nc.vector
                    gi = ti // 8
                    if ti % 8 == 0:
                        g = grpp.tile([P, QCH], BF16, tag="grp",
                                      name="grp")
                        eng.tensor_copy(g, ex)
                        st.grps[h][gi] = g
                    else:
                        g = st.grps[h][gi]
                        eng.tensor_add(g, g, ex)
                elif first:
                    # cq0's first tile: single start=True write for the
                    # whole bank (start resets the bank, so it cannot be
                    # split); waits for the affine mask, but the fine
                    # interleave absorbs that
                    nc.tensor.matmul(
                        st.yaccs[h][:, o:], v_nat[st.b][:, ti],
                        ex[:, o:], start=True, stop=False,
                        skip_group_check=True)
                    st.deferred.append((ti, h, ex, o, last, False))
                else:
                    # diagonal tile: the unmasked suffix can run now;
                    # the masked 128-wide block + denominator defer one
                    # ti-step (see att_drain_deferred)
                    if o + P < QCH:
                        nc.tensor.matmul(
                            st.yaccs[h][:, o + P:],
                            v_nat[st.b][:, ti], ex[:, o + P:],
                            start=False, stop=False,
                            skip_group_check=True)
                    st.deferred.append((ti, h, ex, o, last, True))
                if last:
                    att_drain_deferred(st)
                    # close the denominator (group matmuls) and start
                    # this head's normalize chain immediately
                    for gi in range(len(st.grps[h])):
                        nc.tensor.matmul(
                            st.daccs[h], ones_col, st.grps[h][gi],
                            start=False,
                            stop=(gi == len(st.grps[h]) - 1),
                            skip_group_check=True)
                    rec_f = smallp.tile([1, QCH], F32, tag="rec_f",
                                        name="rec_f")
                    nc.vector.reciprocal(rec_f, st.daccs[h])
                    bcs = workp.tile([P, QCH], F32, tag="bcast",
                                     name="bcast")
                    nc.gpsimd.partition_broadcast(bcs, rec_f)
                    # m = 8*y (fp32) -> hi (Pool) + lo (DVE) fp8 pair
                    mn = workp.tile([P, QCH], F32, tag="mnorm",
                                    name="mnorm")
                    nc.vector.tensor_mul(mn, st.yaccs[h], bcs)
                    nc.gpsimd.tensor_copy(st.ytch[:, h], mn)
                    nc.vector.tensor_sub(st.ytcl[:, h], mn,
                                         st.ytch[:, h])

            def attention_steps(b, cq, fills=0, prelude=None,
                                fine=False):
                """Yields after each sub-phase; last yield is the state
                (with .ytc set). fine=True yields after every single
                emission (for interleaving into phase A). With fills,
                pulls `fills` items/ti from the global queue, and scores
                are software-pipelined one ti-step ahead."""
                st = att_begin(b, cq)
                if not fine:
                    att_sps(st, 0, 0)
                    att_sps(st, 0, 1)
                    yield None
                for ti in range(st.ntk):
                    if fine:
                        att_sps(st, ti, 0)
                        yield None
                        att_sps(st, ti, 1)
                        yield None
                    if prelude is not None:
                        next(prelude, None)
                    if fills:
                        # ti=0 pulls would hit items whose normalize
                        # chain just started; defer them to the tail
                        if ti == 0:
                            pass
                        elif ti >= st.ntk - 2:
                            fill_pull(fills + 1)
                        else:
                            fill_pull(fills)
                    if not fine and ti + 1 < st.ntk:
                        att_sps(st, ti + 1, 0)
                        att_sps(st, ti + 1, 1)
                    if ti < st.ntk - 1:
                        att_drain_deferred(st)
                    att_fin(st, ti, 0)
                    if fine:
                        yield None
                    att_fin(st, ti, 1)
                    yield None
                yield st

            # ---------------- phase A ----------------
            def phase_a_prologue(b, c):
                gc = b * NCH + c
                if b == 0 and c + 1 < NCH:
                    t1 = (c + 1) * TCH
                    nc.sync.dma_start(cs_sb[:, t1:t1 + TCH],
                                      cs[:, t1:t1 + TCH])
                    nc.sync.dma_start(sn_sb[:, t1:t1 + TCH],
                                      sn[:, t1:t1 + TCH])
                if b == 0 and c == NCH - 1:
                    for ko in range(NQ):
                        nc.sync.dma_start(wp_sb[:, ko], wp_r[:, ko])

            def mm1_steps(b, c, first_chunk=False):
                """Generator: per n-step, the 16-ko MM1 chain + evict."""
                phase_a_prologue(b, c)
                gc = b * NCH + c
                xt_sb = xt_tiles.pop(gc)
                raw2 = workp.tile([P, NQ, TCH], BF16, tag="raw2",
                                  name="raw2")
                rawk = workp.tile([P, TCH], BF16, tag="rawk", name="rawk")
                vTc = workp.tile([P, TCH], BF16, tag="vTc", name="vTc")
                tmps = {}

                def evict(n, acc):
                    if n < NQ:
                        nc.scalar.copy(raw2[:, n], acc)
                        src_ap = raw2
                        tmps[n] = workp.tile([P, TCH], BF16, tag="tmp",
                                             name="tmp", bufs=6)
                        nc.sync.dma_start(tmps[n][:P // 2],
                                          raw2[P // 2:, n])
                        nc.sync.dma_start(tmps[n][P // 2:],
                                          raw2[:P // 2, n])
                    elif n == NQ:
                        nc.scalar.copy(rawk, acc)
                        tmps[n] = workp.tile([P, TCH], BF16, tag="tmp",
                                             name="tmp", bufs=6)
                        nc.sync.dma_start(tmps[n][:P // 2],
                                          rawk[P // 2:])
                        nc.sync.dma_start(tmps[n][P // 2:],
                                          rawk[:P // 2])
                    else:
                        nc.scalar.copy(vTc, acc)

                if first_chunk:
                    # ko-group-major over the 4 flex banks: PE starts
                    # after the first 2-ko slice of the wa/xt DMA.
                    accs = [mm_tile() for _ in range(NT)]
                    for (g0, g1) in C0G:
                        for n in range(NT):
                            for kp in range(g0 // 2, g1 // 2):
                                mm1_mms(accs[n], kp, n, xt_sb,
                                        start=(kp == 0),
                                        stop=(kp == KP - 1), skip=True)
                        yield None
                    for n in range(NT):
                        evict(n, accs[n])
                else:
                    for n in range(NT):
                        acc = mm_tile()
                        for kp in range(KP):
                            mm1_mms(acc, kp, n, xt_sb,
                                    start=(kp == 0),
                                    stop=(kp == KP - 1))
                        evict(n, acc)
                        yield None
                # xt prefetch issued AFTER this chunk's evict/tmp DMAs:
                # keeps the small latency-critical transfers (tmps, fp8
                # planes) ahead of the next 2MB stream in the queues
                if gc + 2 < GCN:
                    emit_xt_dma(gc + 2)
                yield (raw2, rawk, vTc, tmps)

            def rope_steps(b, c, raw2, rawk, vTc, tmps):
                """Generator: 4 steps: rope(q0), rope(q1), rope(k),
                v transposes. The half-swapped tmps were produced by
                DMA right after each eviction in mm1_steps."""
                t0 = c * TCH

                def rope_one(src, tmp, dst):
                    t2 = workp.tile([P, TCH], BF16, tag="t2", name="t2")
                    nc.vector.tensor_mul(t2, tmp, sn_sb[:, t0:t0 + TCH])
                    t1 = workp.tile([P, TCH], BF16, tag="t1", name="t1")
                    nc.vector.tensor_mul(t1, src, cs_sb[:, t0:t0 + TCH])
                    tb = workp.tile([P, TCH], BF16, tag="tb", name="tb",
                                    bufs=4)
                    nc.vector.tensor_add(tb, t1, t2)
                    # fp8 convert (Pool) + partition half-swap into the
                    # hd-split [64, 2, T] plane (2 SBUF->SBUF DMAs)
                    f8p = workp.tile([P, TCH], F8, tag="f8p", name="f8p",
                                     bufs=4)
                    nc.gpsimd.tensor_copy(f8p, tb)
                    nc.sync.dma_start(dst[:, 0, :], f8p[:P // 2])
                    nc.sync.dma_start(dst[:, 1, :], f8p[P // 2:])

                for h in range(NQ):
                    rope_one(raw2[:, h], tmps[h], q_planes[b][h][c])
                    yield None
                rope_one(rawk, tmps[NQ], k_plane[b][c])
                yield None
                for i in range(KTQ):
                    pt = mm_tile().bitcast(BF16)[:, :P]
                    nc.tensor.transpose(
                        pt, vTc[:, i * P:(i + 1) * P], ident)
                    nc.vector.tensor_copy(v_nat[b][:, t0 // P + i], pt)
                    if i % 2 == 1:
                        yield None

            def phase_a_batch(b, start_slot, pull_aux, fills=False):
                """Run phase A of batch b with chunk-pipelined rope and
                aux pulls (2 per slot from start_slot on). Returns the
                final chunk's rope generator (not drained)."""
                slot = [0]

                def slot_tick():
                    slot[0] += 1
                    if fills:
                        fill_pull(1)
                    if slot[0] >= start_slot:
                        pull_aux()
                        pull_aux()

                prev_rope = None
                for c in range(NCH):
                    mm1 = mm1_steps(b, c, first_chunk=(b == 0 and c == 0))
                    tail = None
                    for v in mm1:
                        if v is not None:
                            tail = v
                            break
                        if prev_rope is not None:
                            next(prev_rope, None)
                        slot_tick()
                    if prev_rope is not None:
                        for _ in prev_rope:
                            slot_tick()
                    prev_rope = rope_steps(b, c, *tail)
                return prev_rope

            # ================= emission schedule =================
            att_cq0 = {}
            att_cq0_done = {}

            def make_cq0_puller(b):
                gen = attention_steps(b, 0, fine=True)
                att_cq0[b] = gen

                def pull():
                    v = next(gen, False)
                    if v is not False and v is not None:
                        att_cq0_done[b] = v
                return pull

            def drain_cq0(b, rope_tail):
                while b not in att_cq0_done:
                    v = next(att_cq0[b], False)
                    if v is False:
                        break
                    if v is not None:
                        att_cq0_done[b] = v
                    if rope_tail is not None:
                        next(rope_tail, None)

            def chain(*its):
                for it in its:
                    yield from it

            def run_attention(b, cq, prelude=None, fills=1):
                st = None
                for v in attention_steps(b, cq, fills=fills,
                                         prelude=prelude):
                    if v is not None:
                        st = v
                fill_push((st.ytch, st.ytcl), b * T + cq * QCH)

            # ---- batch 0 ----
            alloc_planes(0)
            pull0 = make_cq0_puller(0)
            rope_tail = phase_a_batch(0, 14, pull0)
            drain_cq0(0, rope_tail)
            fill_push((att_cq0_done[0].ytch, att_cq0_done[0].ytcl), 0)

            run_attention(0, 1, prelude=rope_tail)
            run_attention(0, 2)
            run_attention(0, 3)

            # ---- batch 1 ----
            alloc_planes(1)
            pull1 = make_cq0_puller(1)
            rope_tail = phase_a_batch(1, 10, pull1, fills=True)
            drain_cq0(1, rope_tail)
            fill_push((att_cq0_done[1].ytch, att_cq0_done[1].ytcl), T)

            run_attention(1, 1, prelude=rope_tail)
            run_attention(1, 2)
            run_attention(1, 3)
            while fill_q:
                emit_mm4(*fill_q.pop(0), half_dma=True)

    nc.finalize()
    return nc


def _host_prep(x, w_attn, w_proj, freqs_cos, freqs_sin):
    """Shard + relayout inputs for the 8 cores (head-parallel).

    x/w_attn/w_proj ship as fp8e4 hi+lo pairs (same pow2 scale: hi =
    fp8(a*s), lo = fp8(a*s - hi)), stacked on dim1: [rows, 2, cols]."""
    import ml_dtypes
    BF = ml_dtypes.bfloat16
    F8 = ml_dtypes.float8_e4m3

    def split8(a, sc):
        a = np.ascontiguousarray(a * sc)
        hi = a.astype(F8)
        lo = (a - hi.astype(np.float32)).astype(F8)
        return np.ascontiguousarray(np.stack([hi, lo], axis=1))

    x = np.asarray(x, dtype=np.float32)
    w_attn = np.asarray(w_attn, dtype=np.float32)
    w_proj = np.asarray(w_proj, dtype=np.float32)
    fc = np.asarray(freqs_cos, dtype=np.float32)
    fs = np.asarray(freqs_sin, dtype=np.float32)

    # [C, 2, B*T] fp8 -> pre-chunked [B*T/TCH, C, 2, TCH]
    xt = split8(x.reshape(B * T, C).T, 8.0)
    xt = np.ascontiguousarray(
        xt.reshape(C, 2, B * T // TCH, TCH).transpose(2, 0, 1, 3))
    # head-dim layout for q/k is permuted to even-dims-first so that
    # rotate-half becomes a partition half-swap on device; the rotation
    # sign is folded into the sin tensor (top half negated)
    perm = np.concatenate([np.arange(0, HD, 2), np.arange(1, HD, 2)])
    cs_i = np.repeat(fc, 2, axis=1).T  # [HD, T] interleaved layout
    sn_i = np.repeat(fs, 2, axis=1).T
    # cos/sin carry 2^-10 to descale the 1024x raw q/k during rope
    cs = np.ascontiguousarray(cs_i[perm] / 1024.0).astype(BF)
    sgn = np.where(np.arange(HD) < HD // 2, -1.0, 1.0)[:, None]
    sn = np.ascontiguousarray(sn_i[perm] * sgn / 1024.0).astype(
        np.float32).astype(BF)

    in_maps = []
    for g in range(NCORES):
        q0 = w_attn[:, 2 * g * HD:(2 * g + 1) * HD][:, perm]
        q1 = w_attn[:, (2 * g + 1) * HD:(2 * g + 2) * HD][:, perm]
        k_cols = w_attn[:, NH * HD + g * HD:
                        NH * HD + (g + 1) * HD][:, perm]
        v_cols = w_attn[:, (NH + NKV) * HD + g * HD:
                        (NH + NKV) * HD + (g + 1) * HD]
        wa_g = split8(
            np.concatenate([q0, q1, k_cols, v_cols], axis=1), 128.0)
        wp_g = split8(w_proj[2 * g * HD:(2 * g + 2) * HD, :], 128.0)
        in_maps.append({"xt": xt, "wa": wa_g, "wp": wp_g,
                        "cs": cs, "sn": sn})
    return in_maps


def kernel(x, w_attn, w_proj, freqs_cos, freqs_sin):
    from concourse.bass_utils import run_bass_kernel_spmd

    if "nc" not in _CACHE:
        _CACHE["nc"] = _build_nc()
    nc = _CACHE["nc"]
    in_maps = _host_prep(x, w_attn, w_proj, freqs_cos, freqs_sin)
    res = run_bass_kernel_spmd(nc, in_maps, core_ids=list(range(NCORES)))
    acc = np.zeros((B * T, C), dtype=np.float64)
    for r in res.results:
        acc += np.asarray(r["out"], dtype=np.float64)
    acc *= 1.0 / 1024.0  # fold out the 8x (y) * 128x (w_proj) scales
    return acc.reshape(B, T, C).astype(np.float32)

